# revision 11
# baseline (speedup 1.0000x reference)
"""Trainium2 Bass kernel for nn_CharCTCDecoder: MLP + log_softmax + CTC loss.

Sharding: data-parallel over batch across 8 NeuronCores (4 samples/core).
Device computes per core: 3-layer MLP (feature-major matmul chain on PE),
log_softmax, exp-prob tables P_ext, and a linear-space CTC forward DP
(parity-split states, renorm to e^70 every 8 steps). Host does the trivial
glue: input transpose/weight gather, out_lengths, final per-sample
logaddexp + mean.
"""
import sys

sys.path.insert(0, "/opt/trn_rl_repo")

import numpy as np

import concourse.bass as bass
import concourse.tile as tile
from concourse import bacc, mybir
from concourse.bass_utils import run_bass_kernel_spmd

F32 = mybir.dt.float32
AF = mybir.ActivationFunctionType
ALU = mybir.AluOpType

# problem dims (hardcoded per spec)
B, SSRC, D, STRIDE, V, L = 32, 512, 512, 4, 512, 256
NCORES = 8
BS = B // NCORES          # 4 samples per core
T = SSRC * STRIDE         # 2048 output tokens per sample
BLANK = 1
CHS = 256                 # s_src chunk for the MLP phase
NCH = SSRC // CHS         # chunks per sample
TCH = 64                  # DP time chunk
RENORM = 8
CLOG = 70.0               # renorm ceiling e^CLOG


def build_nc(has_b2: bool):
    nc = bacc.Bacc(None, target_bir_lowering=False)

    # per-core external I/O
    xt_d = nc.dram_tensor("xt", [D, BS * SSRC], F32, kind="ExternalInput")
    wexp_d = nc.dram_tensor("w_exp", [D, STRIDE * D], F32, kind="ExternalInput")
    bexp_d = nc.dram_tensor("b_exp", [STRIDE * D], F32, kind="ExternalInput")
    w1_d = nc.dram_tensor("w1", [D, 2 * D], F32, kind="ExternalInput")
    b1_d = nc.dram_tensor("b1", [2 * D], F32, kind="ExternalInput")
    w2_d = nc.dram_tensor("w2", [2 * D, V], F32, kind="ExternalInput")
    w2g_d = nc.dram_tensor("w2g", [2 * D, BS * L], F32, kind="ExternalInput")
    b2e_d = nc.dram_tensor("b2ext", [BS, V + L], F32, kind="ExternalInput")
    g_d = nc.dram_tensor("gmask", [BS, L], F32, kind="ExternalInput")

    lp_d = nc.dram_tensor("logprobs", [BS, T, V], F32, kind="ExternalOutput")
    ee_d = nc.dram_tensor("ee_out", [BS, L + 1], F32, kind="ExternalOutput")
    eo_d = nc.dram_tensor("eo_out", [BS, L], F32, kind="ExternalOutput")
    lacc_d = nc.dram_tensor("lacc_out", [BS, 1], F32, kind="ExternalOutput")

    pext_d = nc.dram_tensor("pext", [BS, T, L + 1], F32)  # internal scratch

    KD = D // 128          # 4 k-chunks for D=512
    KH = 2 * D // 128      # 8 k-chunks for 1024
    NF = STRIDE * D // 128  # 16 statesT feature tiles

    import os as _os
    dbg = bool(_os.environ.get("KDBG"))
    if dbg:
        dbg_st = nc.dram_tensor("dbg_st", [128, NF, CHS], F32, kind="ExternalOutput")
        dbg_h = nc.dram_tensor("dbg_h", [128, KH, STRIDE * CHS], F32,
                               kind="ExternalOutput")
        dbg_lp = nc.dram_tensor("dbg_lp", [128, V], F32, kind="ExternalOutput")

    with tile.TileContext(nc) as tc:
        # ---------------- MLP phase ----------------
        with (
            tc.tile_pool(name="wpool", bufs=1) as wpool,
            tc.tile_pool(name="xpool", bufs=2) as xpool,
            tc.tile_pool(name="spool", bufs=1) as spool,
            tc.tile_pool(name="hpool", bufs=1) as hpool,
            tc.tile_pool(name="lpool", bufs=3) as lpool,
            tc.tile_pool(name="scal", bufs=6) as scal,
            tc.tile_pool(name="mmps", bufs=3, space="PSUM") as mmps,
            tc.tile_pool(name="psA", bufs=2, space="PSUM") as psAp,
            tc.tile_pool(name="psB", bufs=2, space="PSUM") as psBp,
        ):
            wexp_sb = wpool.tile([128, KD, STRIDE * D], F32)
            nc.sync.dma_start(
                out=wexp_sb, in_=wexp_d[:].rearrange("(kc p) f -> p kc f", p=128))
            w1_sb = wpool.tile([128, KD, 2 * D], F32)
            nc.sync.dma_start(
                out=w1_sb, in_=w1_d[:].rearrange("(kc p) f -> p kc f", p=128))
            w2_sb = wpool.tile([128, KH, V], F32)
            nc.sync.dma_start(
                out=w2_sb, in_=w2_d[:].rearrange("(kc p) f -> p kc f", p=128))
            w2g_sb = wpool.tile([128, KH, BS * L], F32)
            nc.sync.dma_start(
                out=w2g_sb, in_=w2g_d[:].rearrange("(kc p) f -> p kc f", p=128))
            bexp_sb = wpool.tile([128, NF], F32)
            nc.sync.dma_start(
                out=bexp_sb, in_=bexp_d[:].rearrange("(f p) -> p f", p=128))
            b1_sb = wpool.tile([128, KH], F32)
            nc.sync.dma_start(
                out=b1_sb, in_=b1_d[:].rearrange("(f p) -> p f", p=128))
            if has_b2:
                ones_sb = wpool.tile([1, 128], F32)
                nc.vector.memset(ones_sb, 1.0)
                b2e_sb = wpool.tile([1, BS, V + L], F32)
                nc.sync.dma_start(out=b2e_sb, in_=b2e_d[:].rearrange("b f -> 1 b f"))

            xt_ap = xt_d[:].rearrange("(kc p) n -> p kc n", p=128)

            for b in range(BS):
                for ch in range(NCH):
                    n0 = b * SSRC + ch * CHS
                    x_sb = xpool.tile([128, KD, CHS], F32)
                    nc.sync.dma_start(out=x_sb, in_=xt_ap[:, :, n0:n0 + CHS])

                    # L1: statesT[f, s_src] = W_exp.T @ x   (feature-major)
                    st_sb = spool.tile([128, NF, CHS], F32)
                    for ft in range(NF):
                        ps = mmps.tile([128, 512], F32, tag="mm")
                        for kc in range(KD):
                            nc.tensor.matmul(
                                ps[:, :CHS],
                                wexp_sb[:, kc, ft * 128:(ft + 1) * 128],
                                x_sb[:, kc, :],
                                start=(kc == 0), stop=(kc == KD - 1))
                        nc.scalar.activation(
                            out=st_sb[:, ft, :], in_=ps[:, :CHS], func=AF.Identity,
                            bias=bexp_sb[:, ft:ft + 1], scale=1.0)

                    # L2: hT[f2, (st, s_src)] = W1.T @ statesT-slab + relu
                    h_sb = hpool.tile([128, KH, STRIDE * CHS], F32)
                    for f2 in range(KH):
                        for st in range(STRIDE):
                            ps = mmps.tile([128, 512], F32, tag="mm")
                            for kc in range(KD):
                                nc.tensor.matmul(
                                    ps[:, :CHS],
                                    w1_sb[:, kc, f2 * 128:(f2 + 1) * 128],
                                    st_sb[:, st * KD + kc, :],
                                    start=(kc == 0), stop=(kc == KD - 1))
                            nc.scalar.activation(
                                out=h_sb[:, f2, st * CHS:(st + 1) * CHS],
                                in_=ps[:, :CHS], func=AF.Relu,
                                bias=b1_sb[:, f2:f2 + 1], scale=1.0)

                    if dbg and b == 0 and ch == 0:
                        nc.sync.dma_start(out=dbg_st[:], in_=st_sb)
                        nc.sync.dma_start(out=dbg_h[:], in_=h_sb)

                    # L3 + log_softmax + P tables, per 128-token tile
                    ntt = STRIDE * CHS // 128
                    for tt in range(ntt):
                        psA = psAp.tile([128, V], F32)
                        psB = psBp.tile([128, L], F32)
                        for kc in range(KH):
                            lhsT = h_sb[:, kc, tt * 128:(tt + 1) * 128]
                            nc.tensor.matmul(
                                psA, lhsT, w2_sb[:, kc, :],
                                start=(kc == 0), stop=(kc == KH - 1 and not has_b2))
                            nc.tensor.matmul(
                                psB, lhsT, w2g_sb[:, kc, b * L:(b + 1) * L],
                                start=(kc == 0), stop=(kc == KH - 1 and not has_b2))
                        if has_b2:
                            nc.tensor.matmul(
                                psA, ones_sb, b2e_sb[:, b, 0:V],
                                start=False, stop=True)
                            nc.tensor.matmul(
                                psB, ones_sb, b2e_sb[:, b, V:V + L],
                                start=False, stop=True)

                        nmax = scal.tile([128, 1], F32)
                        nc.vector.tensor_reduce(
                            out=nmax, in_=psA, axis=mybir.AxisListType.X,
                            op=ALU.max, negate=True)
                        escr = lpool.tile([128, V], F32, tag="escr")
                        sume = scal.tile([128, 1], F32)
                        nc.scalar.activation(
                            out=escr, in_=psA, func=AF.Exp, bias=nmax, scale=1.0,
                            accum_out=sume)
                        lse = scal.tile([128, 1], F32)
                        nc.scalar.activation(out=lse, in_=sume, func=AF.Ln)
                        nl = scal.tile([128, 1], F32)
                        nc.vector.tensor_tensor(
                            out=nl, in0=nmax, in1=lse, op=ALU.subtract)

                        lp_sb = lpool.tile([128, V], F32, tag="lp")
                        nc.vector.tensor_scalar(
                            out=lp_sb, in0=psA, scalar1=nl, scalar2=None,
                            op0=ALU.add)
                        lpt_sb = lpool.tile([128, L], F32, tag="lpt")
                        nc.vector.tensor_scalar(
                            out=lpt_sb, in0=psB, scalar1=nl, scalar2=None,
                            op0=ALU.add)
                        pext_sb = lpool.tile([128, L + 1], F32, tag="pext")
                        nc.scalar.activation(
                            out=pext_sb[:, 0:1], in_=lp_sb[:, BLANK:BLANK + 1],
                            func=AF.Exp)
                        nc.scalar.activation(
                            out=pext_sb[:, 1:L + 1], in_=lpt_sb, func=AF.Exp)

                        if dbg and b == 0 and ch == 0 and tt == 0:
                            nc.sync.dma_start(out=dbg_lp[:], in_=lp_sb)

                        # tokens of this tile: st = tt // (CHS//128) fixed,
                        # s_src = ch*CHS + (tt % (CHS//128))*128 + row
                        st = tt // (CHS // 128)
                        s0 = ch * CHS + (tt % (CHS // 128)) * 128
                        lp_view = lp_d[:].rearrange(
                            "bb (ss st) v -> bb ss st v", st=STRIDE)
                        nc.sync.dma_start(
                            out=lp_view[b, s0:s0 + 128, st, :], in_=lp_sb)
                        pe_view = pext_d[:].rearrange(
                            "bb (ss st) j -> bb ss st j", st=STRIDE)
                        nc.sync.dma_start(
                            out=pe_view[b, s0:s0 + 128, st, :], in_=pext_sb)

        # ---------------- CTC DP phase ----------------
        with (
            tc.tile_pool(name="dpstate", bufs=1) as dps,
            tc.tile_pool(name="dpp", bufs=2) as dpp,
            tc.tile_pool(name="dpscr", bufs=2) as dscr,
            tc.tile_pool(name="dpsc", bufs=4) as dsc,
        ):
            g_sb = dps.tile([BS, L], F32)
            nc.sync.dma_start(out=g_sb, in_=g_d[:])
            ee = [dps.tile([BS, L + 1], F32, tag=f"ee{i}", name=f"ee{i}")
                  for i in range(2)]
            eo = [dps.tile([BS, L + 1], F32, tag=f"eo{i}", name=f"eo{i}")
                  for i in range(2)]
            lacc = dps.tile([BS, 1], F32)
            nc.vector.memset(lacc, 0.0)
            for i in range(2):
                nc.vector.memset(ee[i], 0.0)
                nc.vector.memset(eo[i], 0.0)

            cur = 0
            for tch in range(T // TCH):
                p_sb = dpp.tile([BS, TCH, L + 1], F32)
                nc.sync.dma_start(
                    out=p_sb, in_=pext_d[:, tch * TCH:(tch + 1) * TCH, :])
                for tr in range(TCH):
                    t = tch * TCH + tr
                    pb = p_sb[:, tr, 0:1]
                    pt = p_sb[:, tr, 1:L + 1]
                    if t == 0:
                        # E init: Ee[0] = Pb[0]; Eo[j=0] = Ptgt[0,0]
                        nc.vector.tensor_copy(out=ee[0][:, 0:1], in_=pb)
                        nc.vector.tensor_copy(out=eo[0][:, 1:2], in_=p_sb[:, 0, 1:2])
                        continue
                    es, os_ = ee[cur], eo[cur]
                    ed, od = ee[1 - cur], eo[1 - cur]
                    ue = dscr.tile([BS, L + 1], F32, tag="ue")
                    nc.vector.tensor_tensor(
                        out=ue, in0=es, in1=os_[:, 0:L + 1], op=ALU.add)
                    uo = dscr.tile([BS, L], F32, tag="uo")
                    nc.vector.tensor_tensor(
                        out=uo, in0=os_[:, 1:L + 1], in1=es[:, 0:L], op=ALU.add)
                    w = dscr.tile([BS, L], F32, tag="w")
                    nc.vector.tensor_tensor(
                        out=w, in0=g_sb, in1=os_[:, 0:L], op=ALU.mult)
                    vo = dscr.tile([BS, L], F32, tag="vo")
                    nc.vector.tensor_tensor(out=vo, in0=uo, in1=w, op=ALU.add)
                    nc.vector.tensor_scalar(
                        out=ed, in0=ue, scalar1=pb, scalar2=None, op0=ALU.mult)
                    nc.vector.tensor_tensor(
                        out=od[:, 1:L + 1], in0=vo, in1=pt, op=ALU.mult)
                    cur = 1 - cur

                    if t % RENORM == RENORM - 1:
                        re_ = dsc.tile([BS, 1], F32, tag="re")
                        ro_ = dsc.tile([BS, 1], F32, tag="ro")
                        nc.vector.tensor_reduce(
                            out=re_, in_=ed, axis=mybir.AxisListType.X, op=ALU.max)
                        nc.vector.tensor_reduce(
                            out=ro_, in_=od[:, 1:L + 1],
                            axis=mybir.AxisListType.X, op=ALU.max)
                        mx = dsc.tile([BS, 1], F32, tag="mx")
                        nc.vector.tensor_tensor(
                            out=mx, in0=re_, in1=ro_, op=ALU.max)
                        rinv = dsc.tile([BS, 1], F32, tag="rinv")
                        nc.vector.reciprocal(out=rinv, in_=mx)
                        nc.vector.tensor_scalar(
                            out=ed, in0=ed, scalar1=rinv,
                            scalar2=float(np.exp(CLOG)), op0=ALU.mult, op1=ALU.mult)
                        nc.vector.tensor_scalar(
                            out=od[:, 1:L + 1], in0=od[:, 1:L + 1], scalar1=rinv,
                            scalar2=float(np.exp(CLOG)), op0=ALU.mult, op1=ALU.mult)
                        lml = dsc.tile([BS, 1], F32, tag="lml")
                        nc.scalar.activation(out=lml, in_=mx, func=AF.Ln)
                        nc.vector.tensor_scalar(
                            out=lacc, in0=lacc, scalar1=lml, scalar2=-CLOG,
                            op0=ALU.add, op1=ALU.add)

            nc.sync.dma_start(out=ee_d[:], in_=ee[cur])
            nc.sync.dma_start(out=eo_d[:], in_=eo[cur][:, 1:L + 1])
            nc.sync.dma_start(out=lacc_d[:], in_=lacc)

    nc.compile()
    return nc


_nc_cache = {}


def _get_nc(has_b2):
    if has_b2 not in _nc_cache:
        _nc_cache[has_b2] = build_nc(has_b2)
    return _nc_cache[has_b2]


def kernel(**inputs):
    rep = np.ascontiguousarray(np.asarray(inputs["representation"], np.float32))
    enc = np.asarray(inputs["encoder_mask"])
    tgt = np.asarray(inputs["targets"]).astype(np.int64)
    tmask = np.asarray(inputs["target_mask"])
    W_exp = np.ascontiguousarray(np.asarray(inputs["W_exp"], np.float32))
    b_exp = np.ascontiguousarray(np.asarray(inputs["b_exp"], np.float32))
    W1 = np.ascontiguousarray(np.asarray(inputs["W1"], np.float32))
    b1 = np.ascontiguousarray(np.asarray(inputs["b1"], np.float32))
    W2 = np.ascontiguousarray(np.asarray(inputs["W2"], np.float32))
    b2 = np.ascontiguousarray(np.asarray(inputs["b2"], np.float32))

    seq_lengths = enc.astype(np.int32).sum(1)
    out_lengths = (np.int32(STRIDE) * seq_lengths).astype(np.int32)
    tl = tmask.astype(np.int64).sum(1)
    assert np.all(out_lengths == T), "kernel assumes full encoder mask"

    has_b2 = bool(np.any(b2 != 0))
    nc = _get_nc(has_b2)

    in_maps = []
    for c in range(NCORES):
        bs = slice(c * BS, (c + 1) * BS)
        xt = np.ascontiguousarray(rep[bs].reshape(BS * SSRC, D).T)
        w2g = np.ascontiguousarray(W2[:, tgt[bs]].reshape(2 * D, BS * L))
        g = np.ones((BS, L), np.float32)
        g[:, 1:] = (tgt[bs][:, 1:] != tgt[bs][:, :-1]).astype(np.float32)
        b2e = np.ascontiguousarray(
            np.concatenate([np.broadcast_to(b2, (BS, V)), b2[tgt[bs]]], axis=1)
        ).astype(np.float32)
        in_maps.append({
            "xt": xt,
            "w_exp": W_exp, "b_exp": b_exp,
            "w1": W1, "b1": b1,
            "w2": W2, "w2g": w2g, "b2ext": b2e,
            "gmask": g,
        })

    res = run_bass_kernel_spmd(nc, in_maps, core_ids=list(range(NCORES)))
    global _last_res
    _last_res = res

    logprobs = np.concatenate([r["logprobs"] for r in res.results], axis=0)
    ee = np.concatenate([r["ee_out"] for r in res.results], axis=0)
    eo = np.concatenate([r["eo_out"] for r in res.results], axis=0)
    lacc = np.concatenate([r["lacc_out"] for r in res.results], axis=0)[:, 0]

    with np.errstate(divide="ignore"):
        per = np.empty((B,), np.float64)
        for b in range(B):
            a_last = np.log(np.float64(ee[b, tl[b]])) + np.float64(lacc[b])
            if tl[b] > 0:
                a_prev = np.log(np.float64(eo[b, tl[b] - 1])) + np.float64(lacc[b])
            else:
                a_prev = -np.inf
            per[b] = -np.logaddexp(a_last, a_prev)
    loss = np.float32(np.mean(per / np.maximum(tl, 1)))
    if np.isnan(loss) or np.isinf(loss):
        loss = np.float32(0.0)
    return logprobs, out_lengths, loss


# revision 19
# speedup vs baseline: 1.2013x; 1.2013x over previous
"""Trainium2 Bass kernel for nn_CharCTCDecoder: MLP + log_softmax + CTC loss.

Sharding: data-parallel over batch across 8 NeuronCores (4 samples/core).
Device computes per core: 3-layer MLP (feature-major matmul chain on PE),
log_softmax, exp-prob tables P_ext, and a linear-space CTC forward DP
(parity-split states, renorm to e^70 every 8 steps). Host does the trivial
glue: input transpose/weight gather, out_lengths, final per-sample
logaddexp + mean.
"""
import sys

sys.path.insert(0, "/opt/trn_rl_repo")

import numpy as np

import concourse.bass as bass
import concourse.tile as tile
from concourse import bacc, mybir
from concourse.bass_utils import run_bass_kernel_spmd

F32 = mybir.dt.float32
AF = mybir.ActivationFunctionType
ALU = mybir.AluOpType

# problem dims (hardcoded per spec)
B, SSRC, D, STRIDE, V, L = 32, 512, 512, 4, 512, 256
NCORES = 8
BS = B // NCORES          # 4 samples per core
T = SSRC * STRIDE         # 2048 output tokens per sample
BLANK = 1
CHS = 256                 # s_src chunk for the MLP phase
NCH = SSRC // CHS         # chunks per sample
TCH = 64                  # DP time chunk
RENORM = 8
CLOG = 70.0               # renorm ceiling e^CLOG


def build_nc(has_b2: bool):
    nc = bacc.Bacc(None, target_bir_lowering=False)

    # per-core external I/O
    xt_d = nc.dram_tensor("xt", [D, BS * SSRC], F32, kind="ExternalInput")
    wexp_d = nc.dram_tensor("w_exp", [D, STRIDE * D], F32, kind="ExternalInput")
    bexp_d = nc.dram_tensor("b_exp", [STRIDE * D], F32, kind="ExternalInput")
    w1_d = nc.dram_tensor("w1", [D, 2 * D], F32, kind="ExternalInput")
    b1_d = nc.dram_tensor("b1", [2 * D], F32, kind="ExternalInput")
    w2_d = nc.dram_tensor("w2", [2 * D, V], F32, kind="ExternalInput")
    w2g_d = nc.dram_tensor("w2g", [2 * D, BS * L], F32, kind="ExternalInput")
    b2e_d = nc.dram_tensor("b2ext", [BS, V + L], F32, kind="ExternalInput")
    g_d = nc.dram_tensor("gmask", [BS, L], F32, kind="ExternalInput")

    lp_d = nc.dram_tensor("logprobs", [BS, T, V], F32, kind="ExternalOutput")
    ee_d = nc.dram_tensor("ee_out", [BS, L + 1], F32, kind="ExternalOutput")
    eo_d = nc.dram_tensor("eo_out", [BS, L], F32, kind="ExternalOutput")
    lacc_d = nc.dram_tensor("lacc_out", [BS, 1], F32, kind="ExternalOutput")

    pext_d = nc.dram_tensor("pext", [BS, T, L + 1], F32)  # internal scratch

    KD = D // 128          # 4 k-chunks for D=512
    KH = 2 * D // 128      # 8 k-chunks for 1024
    NF = STRIDE * D // 128  # 16 statesT feature tiles

    import os as _os
    dbg = bool(_os.environ.get("KDBG"))
    if dbg:
        dbg_st = nc.dram_tensor("dbg_st", [128, NF, CHS], F32, kind="ExternalOutput")
        dbg_h = nc.dram_tensor("dbg_h", [128, KH, STRIDE * CHS], F32,
                               kind="ExternalOutput")
        dbg_lp = nc.dram_tensor("dbg_lp", [128, V], F32, kind="ExternalOutput")

    with tile.TileContext(nc) as tc:
        # ---------------- MLP phase ----------------
        with (
            tc.tile_pool(name="wpool", bufs=1) as wpool,
            tc.tile_pool(name="xpool", bufs=2) as xpool,
            tc.tile_pool(name="spool", bufs=1) as spool,
            tc.tile_pool(name="hpool", bufs=1) as hpool,
            tc.tile_pool(name="lpool", bufs=3) as lpool,
            tc.tile_pool(name="scal", bufs=6) as scal,
            tc.tile_pool(name="mmps", bufs=3, space="PSUM") as mmps,
            tc.tile_pool(name="psA", bufs=2, space="PSUM") as psAp,
            tc.tile_pool(name="psB", bufs=2, space="PSUM") as psBp,
        ):
            wexp_sb = wpool.tile([128, KD, STRIDE * D], F32)
            nc.sync.dma_start(
                out=wexp_sb, in_=wexp_d[:].rearrange("(kc p) f -> p kc f", p=128))
            w1_sb = wpool.tile([128, KD, 2 * D], F32)
            nc.sync.dma_start(
                out=w1_sb, in_=w1_d[:].rearrange("(kc p) f -> p kc f", p=128))
            w2_sb = wpool.tile([128, KH, V], F32)
            nc.sync.dma_start(
                out=w2_sb, in_=w2_d[:].rearrange("(kc p) f -> p kc f", p=128))
            w2g_sb = wpool.tile([128, KH, BS * L], F32)
            nc.sync.dma_start(
                out=w2g_sb, in_=w2g_d[:].rearrange("(kc p) f -> p kc f", p=128))
            bexp_sb = wpool.tile([128, NF], F32)
            nc.sync.dma_start(
                out=bexp_sb, in_=bexp_d[:].rearrange("(f p) -> p f", p=128))
            b1_sb = wpool.tile([128, KH], F32)
            nc.sync.dma_start(
                out=b1_sb, in_=b1_d[:].rearrange("(f p) -> p f", p=128))
            if has_b2:
                ones_sb = wpool.tile([1, 128], F32)
                nc.vector.memset(ones_sb, 1.0)
                b2e_sb = wpool.tile([1, BS, V + L], F32)
                nc.sync.dma_start(out=b2e_sb, in_=b2e_d[:].rearrange("b f -> 1 b f"))

            xt_ap = xt_d[:].rearrange("(kc p) n -> p kc n", p=128)

            for b in range(BS):
                for ch in range(NCH):
                    n0 = b * SSRC + ch * CHS
                    x_sb = xpool.tile([128, KD, CHS], F32)
                    nc.sync.dma_start(out=x_sb, in_=xt_ap[:, :, n0:n0 + CHS])

                    # L1: statesT[f, s_src] = W_exp.T @ x   (feature-major)
                    st_sb = spool.tile([128, NF, CHS], F32)
                    for ft in range(NF):
                        ps = mmps.tile([128, 512], F32, tag="mm")
                        for kc in range(KD):
                            nc.tensor.matmul(
                                ps[:, :CHS],
                                wexp_sb[:, kc, ft * 128:(ft + 1) * 128],
                                x_sb[:, kc, :],
                                start=(kc == 0), stop=(kc == KD - 1))
                        nc.scalar.activation(
                            out=st_sb[:, ft, :], in_=ps[:, :CHS], func=AF.Identity,
                            bias=bexp_sb[:, ft:ft + 1], scale=1.0)

                    # L2: hT[f2, (st, s_src)] = W1.T @ statesT-slab + relu
                    h_sb = hpool.tile([128, KH, STRIDE * CHS], F32)
                    for f2 in range(KH):
                        for st in range(STRIDE):
                            ps = mmps.tile([128, 512], F32, tag="mm")
                            for kc in range(KD):
                                nc.tensor.matmul(
                                    ps[:, :CHS],
                                    w1_sb[:, kc, f2 * 128:(f2 + 1) * 128],
                                    st_sb[:, st * KD + kc, :],
                                    start=(kc == 0), stop=(kc == KD - 1))
                            nc.scalar.activation(
                                out=h_sb[:, f2, st * CHS:(st + 1) * CHS],
                                in_=ps[:, :CHS], func=AF.Relu,
                                bias=b1_sb[:, f2:f2 + 1], scale=1.0)

                    if dbg and b == 0 and ch == 0:
                        nc.sync.dma_start(out=dbg_st[:], in_=st_sb)
                        nc.sync.dma_start(out=dbg_h[:], in_=h_sb)

                    # L3 + log_softmax + P tables, per 128-token tile
                    ntt = STRIDE * CHS // 128
                    for tt in range(ntt):
                        psA = psAp.tile([128, V], F32)
                        psB = psBp.tile([128, L], F32)
                        for kc in range(KH):
                            lhsT = h_sb[:, kc, tt * 128:(tt + 1) * 128]
                            nc.tensor.matmul(
                                psA, lhsT, w2_sb[:, kc, :],
                                start=(kc == 0), stop=(kc == KH - 1 and not has_b2))
                            nc.tensor.matmul(
                                psB, lhsT, w2g_sb[:, kc, b * L:(b + 1) * L],
                                start=(kc == 0), stop=(kc == KH - 1 and not has_b2))
                        if has_b2:
                            nc.tensor.matmul(
                                psA, ones_sb, b2e_sb[:, b, 0:V],
                                start=False, stop=True)
                            nc.tensor.matmul(
                                psB, ones_sb, b2e_sb[:, b, V:V + L],
                                start=False, stop=True)

                        nmax = scal.tile([128, 1], F32)
                        nc.vector.tensor_reduce(
                            out=nmax, in_=psA, axis=mybir.AxisListType.X,
                            op=ALU.max, negate=True)
                        escr = lpool.tile([128, V], F32, tag="escr")
                        sume = scal.tile([128, 1], F32)
                        nc.scalar.activation(
                            out=escr, in_=psA, func=AF.Exp, bias=nmax, scale=1.0,
                            accum_out=sume)
                        lse = scal.tile([128, 1], F32)
                        nc.scalar.activation(out=lse, in_=sume, func=AF.Ln)
                        nl = scal.tile([128, 1], F32)
                        nc.vector.tensor_tensor(
                            out=nl, in0=nmax, in1=lse, op=ALU.subtract)

                        lp_sb = lpool.tile([128, V], F32, tag="lp")
                        nc.vector.tensor_scalar(
                            out=lp_sb, in0=psA, scalar1=nl, scalar2=None,
                            op0=ALU.add)
                        lpt_sb = lpool.tile([128, L], F32, tag="lpt")
                        nc.vector.tensor_scalar(
                            out=lpt_sb, in0=psB, scalar1=nl, scalar2=None,
                            op0=ALU.add)
                        pext_sb = lpool.tile([128, L + 1], F32, tag="pext")
                        nc.scalar.activation(
                            out=pext_sb[:, 0:1], in_=lp_sb[:, BLANK:BLANK + 1],
                            func=AF.Exp)
                        nc.scalar.activation(
                            out=pext_sb[:, 1:L + 1], in_=lpt_sb, func=AF.Exp)

                        if dbg and b == 0 and ch == 0 and tt == 0:
                            nc.sync.dma_start(out=dbg_lp[:], in_=lp_sb)

                        # tokens of this tile: st = tt // (CHS//128) fixed,
                        # s_src = ch*CHS + (tt % (CHS//128))*128 + row
                        st = tt // (CHS // 128)
                        s0 = ch * CHS + (tt % (CHS // 128)) * 128
                        lp_view = lp_d[:].rearrange(
                            "bb (ss st) v -> bb ss st v", st=STRIDE)
                        nc.sync.dma_start(
                            out=lp_view[b, s0:s0 + 128, st, :], in_=lp_sb)
                        pe_view = pext_d[:].rearrange(
                            "bb (ss st) j -> bb ss st j", st=STRIDE)
                        nc.sync.dma_start(
                            out=pe_view[b, s0:s0 + 128, st, :], in_=pext_sb)

        # ---------------- CTC DP phase ----------------
        with (
            tc.tile_pool(name="dpstate", bufs=1) as dps,
            tc.tile_pool(name="dpp", bufs=2) as dpp,
            tc.tile_pool(name="dpscr", bufs=2) as dscr,
            tc.tile_pool(name="dpsc", bufs=4) as dsc,
        ):
            g_sb = dps.tile([BS, L], F32)
            nc.sync.dma_start(out=g_sb, in_=g_d[:])
            ee = [dps.tile([BS, L + 1], F32, tag=f"ee{i}", name=f"ee{i}")
                  for i in range(2)]
            eo = [dps.tile([BS, L + 1], F32, tag=f"eo{i}", name=f"eo{i}")
                  for i in range(2)]
            lacc = dps.tile([BS, 1], F32)
            nc.vector.memset(lacc, 0.0)
            rsc = dps.tile([BS, 1], F32)  # pending renorm scale (C/max)
            nc.vector.memset(rsc, 1.0)
            for i in range(2):
                nc.vector.memset(ee[i], 0.0)
                nc.vector.memset(eo[i], 0.0)

            cur = 0
            for tch in range(T // TCH):
                p_sb = dpp.tile([BS, TCH, L + 1], F32)
                nc.sync.dma_start(
                    out=p_sb, in_=pext_d[:, tch * TCH:(tch + 1) * TCH, :])
                for tr in range(TCH):
                    t = tch * TCH + tr
                    pb = p_sb[:, tr, 0:1]
                    pt = p_sb[:, tr, 1:L + 1]
                    if t == 0:
                        # E init: Ee[0] = Pb[0]; Eo[j=0] = Ptgt[0,0]
                        nc.vector.tensor_copy(out=ee[0][:, 0:1], in_=pb)
                        nc.vector.tensor_copy(out=eo[0][:, 1:2], in_=p_sb[:, 0, 1:2])
                        continue
                    es, os_ = ee[cur], eo[cur]
                    ed, od = ee[1 - cur], eo[1 - cur]
                    # reachable band: states j <= t only (rest still zero)
                    le = min(t + 1, L + 1)
                    lo = min(t + 1, L)
                    ue = dscr.tile([BS, L + 1], F32, tag="ue")
                    nc.vector.tensor_tensor(
                        out=ue[:, 0:le], in0=es[:, 0:le], in1=os_[:, 0:le],
                        op=ALU.add)
                    uo = dscr.tile([BS, L], F32, tag="uo")
                    nc.vector.tensor_tensor(
                        out=uo[:, 0:lo], in0=os_[:, 1:1 + lo], in1=es[:, 0:lo],
                        op=ALU.add)
                    w = dscr.tile([BS, L], F32, tag="w")
                    nc.gpsimd.tensor_tensor(
                        out=w[:, 0:lo], in0=g_sb[:, 0:lo], in1=os_[:, 0:lo],
                        op=ALU.mult)
                    vo = dscr.tile([BS, L], F32, tag="vo")
                    nc.vector.tensor_tensor(
                        out=vo[:, 0:lo], in0=uo[:, 0:lo], in1=w[:, 0:lo],
                        op=ALU.add)
                    if t % RENORM == 0:
                        # apply the pending renorm scale computed last step
                        nc.vector.tensor_scalar(
                            out=ed[:, 0:le], in0=ue[:, 0:le], scalar1=pb,
                            scalar2=rsc, op0=ALU.mult, op1=ALU.mult)
                        nc.vector.scalar_tensor_tensor(
                            out=od[:, 1:1 + lo], in0=vo[:, 0:lo], scalar=rsc,
                            in1=pt[:, 0:lo], op0=ALU.mult, op1=ALU.mult)
                    else:
                        nc.vector.tensor_scalar(
                            out=ed[:, 0:le], in0=ue[:, 0:le], scalar1=pb,
                            scalar2=None, op0=ALU.mult)
                        nc.vector.tensor_tensor(
                            out=od[:, 1:1 + lo], in0=vo[:, 0:lo], in1=pt[:, 0:lo],
                            op=ALU.mult)
                    cur = 1 - cur

                    if t % RENORM == RENORM - 1 and t < T - 1:
                        # compute next scale off the critical path; applied at t+1
                        re_ = dsc.tile([BS, 1], F32, tag="re")
                        ro_ = dsc.tile([BS, 1], F32, tag="ro")
                        nc.vector.tensor_reduce(
                            out=re_, in_=ed, axis=mybir.AxisListType.X, op=ALU.max)
                        nc.vector.tensor_reduce(
                            out=ro_, in_=od[:, 1:L + 1],
                            axis=mybir.AxisListType.X, op=ALU.max)
                        mx = dsc.tile([BS, 1], F32, tag="mx")
                        nc.vector.tensor_tensor(
                            out=mx, in0=re_, in1=ro_, op=ALU.max)
                        rinv = dsc.tile([BS, 1], F32, tag="rinv")
                        nc.vector.reciprocal(out=rinv, in_=mx)
                        if t == RENORM - 1:
                            # first renorm: rinv*C overflows fp32 (values are
                            # still at natural ~e^-44 scale) — scale in place
                            # in two chained stages instead of folding.
                            nc.vector.tensor_scalar(
                                out=ed, in0=ed, scalar1=rinv,
                                scalar2=float(np.exp(CLOG)),
                                op0=ALU.mult, op1=ALU.mult)
                            nc.vector.tensor_scalar(
                                out=od[:, 1:L + 1], in0=od[:, 1:L + 1],
                                scalar1=rinv, scalar2=float(np.exp(CLOG)),
                                op0=ALU.mult, op1=ALU.mult)
                        else:
                            nc.vector.tensor_scalar(
                                out=rsc, in0=rinv, scalar1=float(np.exp(CLOG)),
                                scalar2=None, op0=ALU.mult)
                        lml = dsc.tile([BS, 1], F32, tag="lml")
                        nc.scalar.activation(out=lml, in_=mx, func=AF.Ln)
                        nc.vector.tensor_scalar(
                            out=lacc, in0=lacc, scalar1=lml, scalar2=-CLOG,
                            op0=ALU.add, op1=ALU.add)

            nc.sync.dma_start(out=ee_d[:], in_=ee[cur])
            nc.sync.dma_start(out=eo_d[:], in_=eo[cur][:, 1:L + 1])
            nc.sync.dma_start(out=lacc_d[:], in_=lacc)

    nc.compile()
    return nc


_nc_cache = {}


def _get_nc(has_b2):
    if has_b2 not in _nc_cache:
        _nc_cache[has_b2] = build_nc(has_b2)
    return _nc_cache[has_b2]


def kernel(**inputs):
    rep = np.ascontiguousarray(np.asarray(inputs["representation"], np.float32))
    enc = np.asarray(inputs["encoder_mask"])
    tgt = np.asarray(inputs["targets"]).astype(np.int64)
    tmask = np.asarray(inputs["target_mask"])
    W_exp = np.ascontiguousarray(np.asarray(inputs["W_exp"], np.float32))
    b_exp = np.ascontiguousarray(np.asarray(inputs["b_exp"], np.float32))
    W1 = np.ascontiguousarray(np.asarray(inputs["W1"], np.float32))
    b1 = np.ascontiguousarray(np.asarray(inputs["b1"], np.float32))
    W2 = np.ascontiguousarray(np.asarray(inputs["W2"], np.float32))
    b2 = np.ascontiguousarray(np.asarray(inputs["b2"], np.float32))

    seq_lengths = enc.astype(np.int32).sum(1)
    out_lengths = (np.int32(STRIDE) * seq_lengths).astype(np.int32)
    tl = tmask.astype(np.int64).sum(1)
    assert np.all(out_lengths == T), "kernel assumes full encoder mask"

    has_b2 = bool(np.any(b2 != 0))
    nc = _get_nc(has_b2)

    in_maps = []
    for c in range(NCORES):
        bs = slice(c * BS, (c + 1) * BS)
        xt = np.ascontiguousarray(rep[bs].reshape(BS * SSRC, D).T)
        w2g = np.ascontiguousarray(W2[:, tgt[bs]].reshape(2 * D, BS * L))
        g = np.ones((BS, L), np.float32)
        g[:, 1:] = (tgt[bs][:, 1:] != tgt[bs][:, :-1]).astype(np.float32)
        b2e = np.ascontiguousarray(
            np.concatenate([np.broadcast_to(b2, (BS, V)), b2[tgt[bs]]], axis=1)
        ).astype(np.float32)
        in_maps.append({
            "xt": xt,
            "w_exp": W_exp, "b_exp": b_exp,
            "w1": W1, "b1": b1,
            "w2": W2, "w2g": w2g, "b2ext": b2e,
            "gmask": g,
        })

    res = run_bass_kernel_spmd(nc, in_maps, core_ids=list(range(NCORES)))
    global _last_res
    _last_res = res

    logprobs = np.concatenate([r["logprobs"] for r in res.results], axis=0)
    ee = np.concatenate([r["ee_out"] for r in res.results], axis=0)
    eo = np.concatenate([r["eo_out"] for r in res.results], axis=0)
    lacc = np.concatenate([r["lacc_out"] for r in res.results], axis=0)[:, 0]

    with np.errstate(divide="ignore"):
        per = np.empty((B,), np.float64)
        for b in range(B):
            a_last = np.log(np.float64(ee[b, tl[b]])) + np.float64(lacc[b])
            if tl[b] > 0:
                a_prev = np.log(np.float64(eo[b, tl[b] - 1])) + np.float64(lacc[b])
            else:
                a_prev = -np.inf
            per[b] = -np.logaddexp(a_last, a_prev)
    loss = np.float32(np.mean(per / np.maximum(tl, 1)))
    if np.isnan(loss) or np.isinf(loss):
        loss = np.float32(0.0)
    return logprobs, out_lengths, loss


# revision 21
# speedup vs baseline: 1.2400x; 1.0323x over previous
"""Trainium2 Bass kernel for nn_CharCTCDecoder: MLP + log_softmax + CTC loss.

Sharding: data-parallel over batch across 8 NeuronCores (4 samples/core).
Device computes per core: 3-layer MLP (feature-major matmul chain on PE),
log_softmax, exp-prob tables P_ext, and a linear-space CTC forward DP
(parity-split states, renorm to e^70 every 8 steps). Host does the trivial
glue: input transpose/weight gather, out_lengths, final per-sample
logaddexp + mean.
"""
import sys

sys.path.insert(0, "/opt/trn_rl_repo")

import numpy as np

import concourse.bass as bass
import concourse.tile as tile
from concourse import bacc, mybir
from concourse.bass_utils import run_bass_kernel_spmd

F32 = mybir.dt.float32
AF = mybir.ActivationFunctionType
ALU = mybir.AluOpType

# problem dims (hardcoded per spec)
B, SSRC, D, STRIDE, V, L = 32, 512, 512, 4, 512, 256
NCORES = 8
BS = B // NCORES          # 4 samples per core
T = SSRC * STRIDE         # 2048 output tokens per sample
BLANK = 1
CHS = 256                 # s_src chunk for the MLP phase
NCH = SSRC // CHS         # chunks per sample
TCH = 64                  # DP time chunk
RENORM = 8
CLOG = 70.0               # renorm ceiling e^CLOG


def build_nc(has_b2: bool):
    nc = bacc.Bacc(None, target_bir_lowering=False)

    # per-core external I/O
    xt_d = nc.dram_tensor("xt", [D, BS * SSRC], F32, kind="ExternalInput")
    wexp_d = nc.dram_tensor("w_exp", [D, STRIDE * D], F32, kind="ExternalInput")
    bexp_d = nc.dram_tensor("b_exp", [STRIDE * D], F32, kind="ExternalInput")
    w1_d = nc.dram_tensor("w1", [D, 2 * D], F32, kind="ExternalInput")
    b1_d = nc.dram_tensor("b1", [2 * D], F32, kind="ExternalInput")
    w2_d = nc.dram_tensor("w2", [2 * D, V], F32, kind="ExternalInput")
    w2g_d = nc.dram_tensor("w2g", [2 * D, BS * L], F32, kind="ExternalInput")
    b2e_d = nc.dram_tensor("b2ext", [BS, V + L], F32, kind="ExternalInput")
    g_d = nc.dram_tensor("gmask", [BS, L], F32, kind="ExternalInput")

    lp_d = nc.dram_tensor("logprobs", [BS, T, V], F32, kind="ExternalOutput")
    ee_d = nc.dram_tensor("ee_out", [BS, L + 1], F32, kind="ExternalOutput")
    eo_d = nc.dram_tensor("eo_out", [BS, L], F32, kind="ExternalOutput")
    lacc_d = nc.dram_tensor("lacc_out", [BS, 1], F32, kind="ExternalOutput")

    pext_d = nc.dram_tensor("pext", [BS, T, L + 1], F32)  # internal scratch

    KD = D // 128          # 4 k-chunks for D=512
    KH = 2 * D // 128      # 8 k-chunks for 1024
    NF = STRIDE * D // 128  # 16 statesT feature tiles

    import os as _os
    dbg = bool(_os.environ.get("KDBG"))
    if dbg:
        dbg_st = nc.dram_tensor("dbg_st", [128, NF, CHS], F32, kind="ExternalOutput")
        dbg_h = nc.dram_tensor("dbg_h", [128, KH, STRIDE * CHS], F32,
                               kind="ExternalOutput")
        dbg_lp = nc.dram_tensor("dbg_lp", [128, V], F32, kind="ExternalOutput")

    with tile.TileContext(nc) as tc:
        # ---------------- MLP phase ----------------
        with (
            tc.tile_pool(name="wpool", bufs=1) as wpool,
            tc.tile_pool(name="xpool", bufs=2) as xpool,
            tc.tile_pool(name="spool", bufs=1) as spool,
            tc.tile_pool(name="hpool", bufs=1) as hpool,
            tc.tile_pool(name="lpool", bufs=3) as lpool,
            tc.tile_pool(name="scal", bufs=6) as scal,
            tc.tile_pool(name="mmps", bufs=3, space="PSUM") as mmps,
            tc.tile_pool(name="psA", bufs=2, space="PSUM") as psAp,
            tc.tile_pool(name="psB", bufs=2, space="PSUM") as psBp,
        ):
            wexp_sb = wpool.tile([128, KD, STRIDE * D], F32)
            nc.sync.dma_start(
                out=wexp_sb, in_=wexp_d[:].rearrange("(kc p) f -> p kc f", p=128))
            w1_sb = wpool.tile([128, KD, 2 * D], F32)
            nc.sync.dma_start(
                out=w1_sb, in_=w1_d[:].rearrange("(kc p) f -> p kc f", p=128))
            w2_sb = wpool.tile([128, KH, V], F32)
            nc.sync.dma_start(
                out=w2_sb, in_=w2_d[:].rearrange("(kc p) f -> p kc f", p=128))
            w2g_sb = wpool.tile([128, KH, BS * L], F32)
            nc.sync.dma_start(
                out=w2g_sb, in_=w2g_d[:].rearrange("(kc p) f -> p kc f", p=128))
            bexp_sb = wpool.tile([128, NF], F32)
            nc.sync.dma_start(
                out=bexp_sb, in_=bexp_d[:].rearrange("(f p) -> p f", p=128))
            b1_sb = wpool.tile([128, KH], F32)
            nc.sync.dma_start(
                out=b1_sb, in_=b1_d[:].rearrange("(f p) -> p f", p=128))
            if has_b2:
                ones_sb = wpool.tile([1, 128], F32)
                nc.vector.memset(ones_sb, 1.0)
                b2e_sb = wpool.tile([1, BS, V + L], F32)
                nc.sync.dma_start(out=b2e_sb, in_=b2e_d[:].rearrange("b f -> 1 b f"))

            xt_ap = xt_d[:].rearrange("(kc p) n -> p kc n", p=128)

            for b in range(BS):
                for ch in range(NCH):
                    n0 = b * SSRC + ch * CHS
                    x_sb = xpool.tile([128, KD, CHS], F32)
                    nc.sync.dma_start(out=x_sb, in_=xt_ap[:, :, n0:n0 + CHS])

                    # L1: statesT[f, s_src] = W_exp.T @ x   (feature-major)
                    st_sb = spool.tile([128, NF, CHS], F32)
                    for ft in range(NF):
                        ps = mmps.tile([128, 512], F32, tag="mm")
                        for kc in range(KD):
                            nc.tensor.matmul(
                                ps[:, :CHS],
                                wexp_sb[:, kc, ft * 128:(ft + 1) * 128],
                                x_sb[:, kc, :],
                                start=(kc == 0), stop=(kc == KD - 1))
                        nc.scalar.activation(
                            out=st_sb[:, ft, :], in_=ps[:, :CHS], func=AF.Identity,
                            bias=bexp_sb[:, ft:ft + 1], scale=1.0)

                    # L2: hT[f2, (st, s_src)] = W1.T @ statesT-slab + relu
                    h_sb = hpool.tile([128, KH, STRIDE * CHS], F32)
                    for f2 in range(KH):
                        for st in range(STRIDE):
                            ps = mmps.tile([128, 512], F32, tag="mm")
                            for kc in range(KD):
                                nc.tensor.matmul(
                                    ps[:, :CHS],
                                    w1_sb[:, kc, f2 * 128:(f2 + 1) * 128],
                                    st_sb[:, st * KD + kc, :],
                                    start=(kc == 0), stop=(kc == KD - 1))
                            nc.scalar.activation(
                                out=h_sb[:, f2, st * CHS:(st + 1) * CHS],
                                in_=ps[:, :CHS], func=AF.Relu,
                                bias=b1_sb[:, f2:f2 + 1], scale=1.0)

                    if dbg and b == 0 and ch == 0:
                        nc.sync.dma_start(out=dbg_st[:], in_=st_sb)
                        nc.sync.dma_start(out=dbg_h[:], in_=h_sb)

                    # L3 + log_softmax + P tables, per 128-token tile
                    ntt = STRIDE * CHS // 128
                    for tt in range(ntt):
                        psA = psAp.tile([128, V], F32)
                        psB = psBp.tile([128, L], F32)
                        for kc in range(KH):
                            lhsT = h_sb[:, kc, tt * 128:(tt + 1) * 128]
                            nc.tensor.matmul(
                                psA, lhsT, w2_sb[:, kc, :],
                                start=(kc == 0), stop=(kc == KH - 1 and not has_b2))
                            nc.tensor.matmul(
                                psB, lhsT, w2g_sb[:, kc, b * L:(b + 1) * L],
                                start=(kc == 0), stop=(kc == KH - 1 and not has_b2))
                        if has_b2:
                            nc.tensor.matmul(
                                psA, ones_sb, b2e_sb[:, b, 0:V],
                                start=False, stop=True)
                            nc.tensor.matmul(
                                psB, ones_sb, b2e_sb[:, b, V:V + L],
                                start=False, stop=True)

                        nmax = scal.tile([128, 1], F32)
                        nc.vector.tensor_reduce(
                            out=nmax, in_=psA, axis=mybir.AxisListType.X,
                            op=ALU.max, negate=True)
                        escr = lpool.tile([128, V], F32, tag="escr")
                        sume = scal.tile([128, 1], F32)
                        nc.scalar.activation(
                            out=escr, in_=psA, func=AF.Exp, bias=nmax, scale=1.0,
                            accum_out=sume)
                        lse = scal.tile([128, 1], F32)
                        nc.scalar.activation(out=lse, in_=sume, func=AF.Ln)
                        nl = scal.tile([128, 1], F32)
                        nc.vector.tensor_tensor(
                            out=nl, in0=nmax, in1=lse, op=ALU.subtract)

                        lp_sb = lpool.tile([128, V], F32, tag="lp")
                        nc.vector.tensor_scalar(
                            out=lp_sb, in0=psA, scalar1=nl, scalar2=None,
                            op0=ALU.add)
                        lpt_sb = lpool.tile([128, L], F32, tag="lpt")
                        nc.vector.tensor_scalar(
                            out=lpt_sb, in0=psB, scalar1=nl, scalar2=None,
                            op0=ALU.add)
                        pext_sb = lpool.tile([128, L + 1], F32, tag="pext")
                        nc.scalar.activation(
                            out=pext_sb[:, 0:1], in_=lp_sb[:, BLANK:BLANK + 1],
                            func=AF.Exp)
                        nc.scalar.activation(
                            out=pext_sb[:, 1:L + 1], in_=lpt_sb, func=AF.Exp)

                        if dbg and b == 0 and ch == 0 and tt == 0:
                            nc.sync.dma_start(out=dbg_lp[:], in_=lp_sb)

                        # tokens of this tile: st = tt // (CHS//128) fixed,
                        # s_src = ch*CHS + (tt % (CHS//128))*128 + row
                        st = tt // (CHS // 128)
                        s0 = ch * CHS + (tt % (CHS // 128)) * 128
                        lp_view = lp_d[:].rearrange(
                            "bb (ss st) v -> bb ss st v", st=STRIDE)
                        nc.sync.dma_start(
                            out=lp_view[b, s0:s0 + 128, st, :], in_=lp_sb)
                        pe_view = pext_d[:].rearrange(
                            "bb (ss st) j -> bb ss st j", st=STRIDE)
                        nc.sync.dma_start(
                            out=pe_view[b, s0:s0 + 128, st, :], in_=pext_sb)

        # ---------------- CTC DP phase ----------------
        with (
            tc.tile_pool(name="dpstate", bufs=1) as dps,
            tc.tile_pool(name="dpp", bufs=2) as dpp,
            tc.tile_pool(name="dpscr", bufs=2) as dscr,
            tc.tile_pool(name="dpsc", bufs=4) as dsc,
        ):
            g_sb = dps.tile([BS, L], F32)
            nc.sync.dma_start(out=g_sb, in_=g_d[:])
            ee = [dps.tile([BS, L + 1], F32, tag=f"ee{i}", name=f"ee{i}")
                  for i in range(2)]
            eo = [dps.tile([BS, L + 1], F32, tag=f"eo{i}", name=f"eo{i}")
                  for i in range(2)]
            lacc = dps.tile([BS, 1], F32)
            nc.vector.memset(lacc, 0.0)
            rsc = dps.tile([BS, 1], F32)  # pending renorm scale (C/max)
            nc.vector.memset(rsc, 1.0)
            for i in range(2):
                nc.vector.memset(ee[i], 0.0)
                nc.vector.memset(eo[i], 0.0)

            cur = 0
            for tch in range(T // TCH):
                p_sb = dpp.tile([BS, TCH, L + 1], F32)
                nc.sync.dma_start(
                    out=p_sb, in_=pext_d[:, tch * TCH:(tch + 1) * TCH, :])
                for tr in range(TCH):
                    t = tch * TCH + tr
                    pb = p_sb[:, tr, 0:1]
                    pt = p_sb[:, tr, 1:L + 1]
                    if t == 0:
                        # E init: Ee[0] = Pb[0]; Eo[j=0] = Ptgt[0,0]
                        nc.vector.tensor_copy(out=ee[0][:, 0:1], in_=pb)
                        nc.vector.tensor_copy(out=eo[0][:, 1:2], in_=p_sb[:, 0, 1:2])
                        continue
                    es, os_ = ee[cur], eo[cur]
                    ed, od = ee[1 - cur], eo[1 - cur]
                    # reachable band: j <= t (prefix; rest still zero) and,
                    # near the end, only states that can still reach the
                    # final ones: s + 2*(T-1-t) >= 511 (margin 2 for safety)
                    le = min(t + 1, L + 1)
                    lo = min(t + 1, L)
                    rem = T - 1 - t
                    je = max(0, L - rem - 2)       # even states j >= 256-rem
                    jo = max(0, L - 1 - rem - 2)   # odd states  j >= 255-rem
                    ue = dscr.tile([BS, L + 1], F32, tag="ue")
                    nc.vector.tensor_tensor(
                        out=ue[:, je:le], in0=es[:, je:le], in1=os_[:, je:le],
                        op=ALU.add)
                    uo = dscr.tile([BS, L], F32, tag="uo")
                    nc.vector.tensor_tensor(
                        out=uo[:, jo:lo], in0=os_[:, 1 + jo:1 + lo],
                        in1=es[:, jo:lo], op=ALU.add)
                    w = dscr.tile([BS, L], F32, tag="w")
                    nc.gpsimd.tensor_tensor(
                        out=w[:, jo:lo], in0=g_sb[:, jo:lo], in1=os_[:, jo:lo],
                        op=ALU.mult)
                    vo = dscr.tile([BS, L], F32, tag="vo")
                    nc.vector.tensor_tensor(
                        out=vo[:, jo:lo], in0=uo[:, jo:lo], in1=w[:, jo:lo],
                        op=ALU.add)
                    if t % RENORM == 0:
                        # apply the pending renorm scale computed last step
                        nc.vector.tensor_scalar(
                            out=ed[:, je:le], in0=ue[:, je:le], scalar1=pb,
                            scalar2=rsc, op0=ALU.mult, op1=ALU.mult)
                        nc.vector.scalar_tensor_tensor(
                            out=od[:, 1 + jo:1 + lo], in0=vo[:, jo:lo], scalar=rsc,
                            in1=pt[:, jo:lo], op0=ALU.mult, op1=ALU.mult)
                    else:
                        nc.vector.tensor_scalar(
                            out=ed[:, je:le], in0=ue[:, je:le], scalar1=pb,
                            scalar2=None, op0=ALU.mult)
                        nc.vector.tensor_tensor(
                            out=od[:, 1 + jo:1 + lo], in0=vo[:, jo:lo],
                            in1=pt[:, jo:lo], op=ALU.mult)
                    cur = 1 - cur

                    if t % RENORM == RENORM - 1 and t < T - 1:
                        # compute next scale off the critical path; applied at t+1
                        re_ = dsc.tile([BS, 1], F32, tag="re")
                        ro_ = dsc.tile([BS, 1], F32, tag="ro")
                        nc.vector.tensor_reduce(
                            out=re_, in_=ed[:, je:le], axis=mybir.AxisListType.X,
                            op=ALU.max)
                        nc.vector.tensor_reduce(
                            out=ro_, in_=od[:, 1 + jo:1 + lo],
                            axis=mybir.AxisListType.X, op=ALU.max)
                        mx = dsc.tile([BS, 1], F32, tag="mx")
                        nc.vector.tensor_tensor(
                            out=mx, in0=re_, in1=ro_, op=ALU.max)
                        rinv = dsc.tile([BS, 1], F32, tag="rinv")
                        nc.vector.reciprocal(out=rinv, in_=mx)
                        if t == RENORM - 1:
                            # first renorm: rinv*C overflows fp32 (values are
                            # still at natural ~e^-44 scale) — scale in place
                            # in two chained stages instead of folding.
                            nc.vector.tensor_scalar(
                                out=ed, in0=ed, scalar1=rinv,
                                scalar2=float(np.exp(CLOG)),
                                op0=ALU.mult, op1=ALU.mult)
                            nc.vector.tensor_scalar(
                                out=od[:, 1:L + 1], in0=od[:, 1:L + 1],
                                scalar1=rinv, scalar2=float(np.exp(CLOG)),
                                op0=ALU.mult, op1=ALU.mult)
                        else:
                            nc.vector.tensor_scalar(
                                out=rsc, in0=rinv, scalar1=float(np.exp(CLOG)),
                                scalar2=None, op0=ALU.mult)
                        lml = dsc.tile([BS, 1], F32, tag="lml")
                        nc.scalar.activation(out=lml, in_=mx, func=AF.Ln)
                        nc.vector.tensor_scalar(
                            out=lacc, in0=lacc, scalar1=lml, scalar2=-CLOG,
                            op0=ALU.add, op1=ALU.add)

            nc.sync.dma_start(out=ee_d[:], in_=ee[cur])
            nc.sync.dma_start(out=eo_d[:], in_=eo[cur][:, 1:L + 1])
            nc.sync.dma_start(out=lacc_d[:], in_=lacc)

    nc.compile()
    return nc


_nc_cache = {}


def _get_nc(has_b2):
    if has_b2 not in _nc_cache:
        _nc_cache[has_b2] = build_nc(has_b2)
    return _nc_cache[has_b2]


def kernel(**inputs):
    rep = np.ascontiguousarray(np.asarray(inputs["representation"], np.float32))
    enc = np.asarray(inputs["encoder_mask"])
    tgt = np.asarray(inputs["targets"]).astype(np.int64)
    tmask = np.asarray(inputs["target_mask"])
    W_exp = np.ascontiguousarray(np.asarray(inputs["W_exp"], np.float32))
    b_exp = np.ascontiguousarray(np.asarray(inputs["b_exp"], np.float32))
    W1 = np.ascontiguousarray(np.asarray(inputs["W1"], np.float32))
    b1 = np.ascontiguousarray(np.asarray(inputs["b1"], np.float32))
    W2 = np.ascontiguousarray(np.asarray(inputs["W2"], np.float32))
    b2 = np.ascontiguousarray(np.asarray(inputs["b2"], np.float32))

    seq_lengths = enc.astype(np.int32).sum(1)
    out_lengths = (np.int32(STRIDE) * seq_lengths).astype(np.int32)
    tl = tmask.astype(np.int64).sum(1)
    assert np.all(out_lengths == T), "kernel assumes full encoder mask"

    has_b2 = bool(np.any(b2 != 0))
    nc = _get_nc(has_b2)

    in_maps = []
    for c in range(NCORES):
        bs = slice(c * BS, (c + 1) * BS)
        xt = np.ascontiguousarray(rep[bs].reshape(BS * SSRC, D).T)
        w2g = np.ascontiguousarray(W2[:, tgt[bs]].reshape(2 * D, BS * L))
        g = np.ones((BS, L), np.float32)
        g[:, 1:] = (tgt[bs][:, 1:] != tgt[bs][:, :-1]).astype(np.float32)
        b2e = np.ascontiguousarray(
            np.concatenate([np.broadcast_to(b2, (BS, V)), b2[tgt[bs]]], axis=1)
        ).astype(np.float32)
        in_maps.append({
            "xt": xt,
            "w_exp": W_exp, "b_exp": b_exp,
            "w1": W1, "b1": b1,
            "w2": W2, "w2g": w2g, "b2ext": b2e,
            "gmask": g,
        })

    res = run_bass_kernel_spmd(nc, in_maps, core_ids=list(range(NCORES)))
    global _last_res
    _last_res = res

    logprobs = np.concatenate([r["logprobs"] for r in res.results], axis=0)
    ee = np.concatenate([r["ee_out"] for r in res.results], axis=0)
    eo = np.concatenate([r["eo_out"] for r in res.results], axis=0)
    lacc = np.concatenate([r["lacc_out"] for r in res.results], axis=0)[:, 0]

    with np.errstate(divide="ignore"):
        per = np.empty((B,), np.float64)
        for b in range(B):
            a_last = np.log(np.float64(ee[b, tl[b]])) + np.float64(lacc[b])
            if tl[b] > 0:
                a_prev = np.log(np.float64(eo[b, tl[b] - 1])) + np.float64(lacc[b])
            else:
                a_prev = -np.inf
            per[b] = -np.logaddexp(a_last, a_prev)
    loss = np.float32(np.mean(per / np.maximum(tl, 1)))
    if np.isnan(loss) or np.isinf(loss):
        loss = np.float32(0.0)
    return logprobs, out_lengths, loss
